# revision 45
# baseline (speedup 1.0000x reference)
"""CLAHE/LCN kernel for Trainium2, 8-core data parallel (v2).

Math (per image, 31x31 'same' zero-padded box window):
    S  = box2d(x)   (sum)      Q = box2d(x^2)   (sum)
    mean = S/961, sqmean = Q/961, var = sqmean - mean^2
    out  = 0.2*x + 0.4 + 0.4*tanh(0.25*(x-mean)/sqrt(var))

Box filter on PE: image block X_b as stationary lhsT against a banded
0/1 moving operand computes the column 31-box of X, transposed. Two
fused transpose+box stages restore natural layout with no transposes.

v2 structure (vs the 284us baseline):
  - stage-2 accumulates into 512-wide single-bank PSUM tiles
    (psS bufs=3, psQ bufs=1, stage-1 ps bufs=2x2 banks = 8 banks).
  - stage-2 band carries the 1/961 scale, shared by S and Q paths:
    psS = 0.5*mean, psQ = sqmean.
  - A = mean^2 via ACT Square (the only legal PSUM reader pairing);
    then TWO PE folds with -identity stationaries: psS -= xb (so psS
    becomes 0.5*(mean-x), killing the DVE num pass) and psQ -= A (so
    psQ becomes var, killing the DVE var STT).
  - rc = rsqrt(4*var) = 0.5/std (ACT, reading var straight from PSUM).
  - z = psS * rc = -0.25*norm (DVE, PSUM operand); th = tanh(z) is
    -tanh(0.25 norm), so out = 0.2x - 0.4*th + 0.4 via one custom DVE
    op (OUTAFF: in0*s0 + in1*s1 + imm2) per quarter.
  - xb is a plain f32->bf16 cast (the 0.5 scale lives in band_b and in
    the -0.5 identity); tb = x^2 is the only GPSIMD op.
  - next image's x quarters + xb + tb are emitted before stage-2 so
    stage-1(i+1) PE work is ready the moment stage-2(i) drains.
  - folds/rsqrt staggered one half behind the bands, z two behind, so
    the PE never waits on ACT; Square is emitted before the Q bands.
Measured 249.5us (baseline 286.2us), rel err 1.15e-2 (gate 2e-2).
"""

import threading

import numpy as np
import ml_dtypes

# ---------------------------------------------------------------- constants
B_FULL = 32          # full batch
NCORES = 8
IMGS = B_FULL // NCORES  # images per core
H = W = 1024
P = 128              # partitions
NBLK = H // P        # 8 row blocks per image
NQ = 4               # quarters per image (2 row-tiles each)
KWIN = 31
HALF = KWIN // 2     # 15
AREA_INV = 1.0 / (KWIN * KWIN)  # 1/961

_lock = threading.Lock()
_compiled = None  # (nc, band_np)


def _band_spec():
    """Per h-block b: (lo, hi, offset into packed band array)."""
    spec = []
    off = 0
    for b in range(NBLK):
        lo = max(0, P * b - HALF)
        hi = min(H, P * b + P + HALF + 1)  # 128b+143
        spec.append((lo, hi, off))
        off += hi - lo
    return spec, off


def _band_np():
    spec, total = _band_spec()
    band = np.zeros((P, total), np.float32)
    for b, (lo, hi, off) in enumerate(spec):
        for h in range(P):
            gh = P * b + h
            r0 = max(lo, gh - HALF)
            r1 = min(hi, gh + HALF + 1)
            band[h, off + (r0 - lo): off + (r1 - lo)] = 1.0
    return band


def _mm_segments():
    """Stage-1 segment list for one [128, 1024] output tile, clipped to
    PSUM bank boundaries: (b, s0, s1, band_off, start, stop)."""
    spec, _ = _band_spec()
    per_bank = {0: [], 1: []}
    for b, (lo, hi, off) in enumerate(spec):
        for bank in (0, 1):
            s0 = max(lo, 512 * bank)
            s1 = min(hi, 512 * bank + 512)
            if s1 > s0:
                per_bank[bank].append((b, s0, s1, off + (s0 - lo)))
    out = []
    for bank in (0, 1):
        segs = per_bank[bank]
        for i, (b, s0, s1, boff) in enumerate(segs):
            out.append((b, s0, s1, boff, i == 0, i == len(segs) - 1))
    return out


def _seg_halves():
    """Stage-2 segments for a [128, 512] psum tile covering output cols
    [512k, 512k+512): per k a list of (b, c0, c1, band_off) with
    psum-local columns."""
    spec, _ = _band_spec()
    halves = {}
    for k in (0, 1):
        lo_k, hi_k = 512 * k, 512 * k + 512
        segs = []
        for b, (lo, hi, off) in enumerate(spec):
            s0, s1 = max(lo, lo_k), min(hi, hi_k)
            if s1 > s0:
                segs.append((b, s0 - lo_k, s1 - lo_k, off + (s0 - lo)))
        halves[k] = segs
    return halves


def _qspec():
    """Even-column (stride-2) band layout for the decimated Q path:
    per block b, (lo_e, n_e, off_q) where lo_e is the first even output
    col in the block's support and n_e the even-col count."""
    spec, _ = _band_spec()
    out = []
    off_q = 0
    for b, (lo, hi, off) in enumerate(spec):
        lo_e = lo + (lo & 1)
        n_e = (hi - lo_e + 1) // 2
        out.append((lo_e, n_e, off_q))
        off_q += n_e
    return out, off_q


def _band_q_np():
    """Packed stride-2, 1/961-scaled band for the decimated Q path."""
    band = _band_np()
    spec, _ = _band_spec()
    qspec, total = _qspec()
    bq = np.zeros((P, total), np.float32)
    for b, (lo, hi, off) in enumerate(spec):
        lo_e, n_e, off_q = qspec[b]
        bq[:, off_q:off_q + n_e] = band[:, off + (lo_e - lo): off + (hi - lo): 2]
    return bq * AREA_INV


def _seg_halves_q():
    """Decimated-Q segments: per half k a list of (b, c0, c1, boff) into
    a [P, 256] psum tile; psum-local col j <-> output col 512k + 2j."""
    spec, _ = _band_spec()
    qspec, _ = _qspec()
    halves = {}
    for k in (0, 1):
        lo_k, hi_k = 512 * k, 512 * k + 512
        segs = []
        for b, (lo, hi, off) in enumerate(spec):
            lo_e, n_e, off_q = qspec[b]
            e0 = max(lo_e, lo_k)
            e1 = min(hi, hi_k)
            if e1 > e0:
                c0 = (e0 - lo_k) // 2
                c1 = (e1 - lo_k + 1) // 2
                segs.append((b, c0, c1, off_q + (e0 - lo_e) // 2))
        halves[k] = segs
    return halves


def _register_outaff_op():
    """Custom DVE op: out = in0*s0 + in1*s1 + imm2 — the whole output
    stage (0.2x - 0.4*th + 0.4) in one bf16 2x-mode instruction."""
    import concourse.dve_ops as dve_ops
    from concourse.dve_spec import Spec, Src0, Src1, C0, C1, C2, lower
    from concourse.dve_spec import _has_src1
    from concourse.dve_uop import DveOpSpec

    name = "OUTAFF_CLAHE"
    for op in dve_ops.OPS:
        if op.name == name:
            return op
    spec = Spec(
        body=Src0 * C0 + Src1 * C1 + C2,
        reference=lambda in0, in1, s0, s1, imm2: (
            in0.astype(np.float32) * s0
            + in1.astype(np.float32) * s1 + imm2),
    )
    row = dve_ops._CUSTOM_DVE_ROW_BASE + len(dve_ops.OPS)
    shas = {}
    for ver in ("v3",):
        uops = lower(spec, ver=ver)
        shas[ver] = DveOpSpec(name=name, opcode=row, uops=uops,
                              rd1_en=_has_src1(spec)).sha(ver)
    op = dve_ops.DveOp(name, spec, subdim=False, uops_sha=shas)
    dve_ops.OPS.append(op)
    dve_ops._SUB_OPCODE_FOR_NAME[name] = row
    dve_ops.CUSTOM_DVE_SPECS[name] = op.spec
    return op


def _patch_act_tables():
    """Hollow every table set except the two this kernel uses, so the
    selector maps Copy/Abs_reciprocal_sqrt to one set and Tanh to the
    other. Dict order (set IDs) is unchanged so emitted IDs stay valid."""
    import concourse.bacc as bacc_mod
    if getattr(bacc_mod, "_clahe_tables_patched", False):
        return
    orig = bacc_mod.get_activation_tables
    keep = {"abs_reciprocal_sqrt_and_small", "silu_and_others"}

    def patched(arch):
        tabs = dict(orig(arch))
        for k in tabs:
            if k not in keep:
                tabs[k] = set()
        return tabs

    bacc_mod.get_activation_tables = patched
    bacc_mod._clahe_tables_patched = True


def y_rows(dram_ap, row0):
    """DRAM AP view [P, 2, W]: element (p, t, c) <-> dram[row0+128t+p, c]."""
    sl = dram_ap[row0: row0 + 256, :]
    return sl.rearrange("(t p) c -> p t c", p=P)


def _build():
    import concourse.bacc as bacc
    import concourse.tile as tile
    from concourse.tile import add_dep_helper
    from concourse import mybir

    _patch_act_tables()
    outaff = _register_outaff_op()

    f32 = mybir.dt.float32
    f16 = mybir.dt.float16
    bf16 = mybir.dt.bfloat16
    ALU = mybir.AluOpType
    ACT = mybir.ActivationFunctionType

    spec, band_w = _band_spec()
    halves = _seg_halves()
    halves_q = _seg_halves_q()
    _, bandq_w = _qspec()
    c = AREA_INV

    nc = bacc.Bacc("TRN2", target_bir_lowering=False, debug=False,
                   num_devices=NCORES)
    x_ext = nc.dram_tensor("x", [IMGS * H, W], f32, kind="ExternalInput")
    bandb_ext = nc.dram_tensor("bandb", [P, band_w], bf16, kind="ExternalInput")
    bandh_ext = nc.dram_tensor("bandh", [P, band_w], f16, kind="ExternalInput")
    bandc_ext = nc.dram_tensor("bandc", [P, band_w], f16, kind="ExternalInput")
    bandq_ext = nc.dram_tensor("bandq", [P, bandq_w], f16, kind="ExternalInput")
    ident_ext = nc.dram_tensor("ident", [P, P], bf16, kind="ExternalInput")
    identa_ext = nc.dram_tensor("identa", [P, P], bf16, kind="ExternalInput")
    y_ext = nc.dram_tensor("y", [IMGS * H, W], bf16, kind="ExternalOutput")
    x_ap = x_ext.ap()
    y_ap = y_ext.ap()

    with tile.TileContext(nc) as tc:
        from contextlib import ExitStack
        with ExitStack() as ctx:
            def pool(name, bufs, space="SBUF"):
                return ctx.enter_context(
                    tc.tile_pool(name=name, bufs=bufs, space=space))

            singles = pool("singles", 1)
            p_x = pool("p_x", 2)       # x quarters [P,2,W] f32
            p_xb = pool("p_xb", 2)     # 0.5x full image [P,8,W] bf16
            p_tb = pool("p_tb", 2)     # x^2 full image [P,8,W] f16
            p_t1 = pool("p_t1", 2)     # t1x/t1t [P,8,W] f16 (shared tag pool)
            p_a = pool("p_a", 3)       # mean^2 per half [P,512] bf16
            p_rcp = pool("p_rcp", 4)   # 0.5/std per half [P,512] bf16
            p_z = pool("p_z", 2)       # z full image [P,8,W] bf16
            p_thu = pool("p_thu", 4)   # tanh quarters [P,2,W] bf16
            p_out = pool("p_out", 2)   # out quarters [P,2,W] bf16
            ps_1 = pool("ps1", 2, space="PSUM")   # [P,512] = 1 bank each
            ps_s = pool("psS", 4, space="PSUM")   # [P,512]
            ps_q = pool("psQ", 2, space="PSUM")   # [P,512]

            band_b = singles.tile([P, band_w], bf16)
            band_h = singles.tile([P, band_w], f16)
            band_c = singles.tile([P, band_w], f16)
            band_q = singles.tile([P, bandq_w], f16)
            ident = singles.tile([P, P], bf16)    # diag(-0.5), num fold
            identa = singles.tile([P, P], bf16)   # diag(-1.0), A fold
            nc.sync.dma_start(out=band_b[:], in_=bandb_ext.ap())
            nc.sync.dma_start(out=band_h[:], in_=bandh_ext.ap())
            nc.sync.dma_start(out=band_c[:], in_=bandc_ext.ap())
            nc.sync.dma_start(out=band_q[:], in_=bandq_ext.ap())
            nc.sync.dma_start(out=ident[:], in_=ident_ext.ap())
            nc.sync.dma_start(out=identa[:], in_=identa_ext.ap())

            def half_mms(ps, band_sb, stat_slicer, k, seg_tab=None):
                segs = (seg_tab or halves)[k]
                n = len(segs)
                for i, (b, c0, c1, boff) in enumerate(segs):
                    nc.tensor.matmul(
                        ps[:, c0:c1],
                        stat_slicer(b),
                        band_sb[:, boff: boff + (c1 - c0)],
                        start=(i == 0), stop=(i == n - 1),
                    )

            # ---------------- per-image tail ---------------------------
            def tail_tanh(img_state, qs, pin=None):
                """tanh for quarters qs; pinned after `pin` (an ACT inst)
                to keep the two ACT table sets from thrashing."""
                xb, z, base, last_rsq, th_tiles = img_state
                for q in qs:
                    th = p_thu.tile([P, 2, W], bf16, tag="thu")
                    th_i = nc.scalar.activation(th[:], z[:, 2 * q:2 * q + 2, :],
                                                ACT.Tanh, bias=0.0, scale=1.0)
                    anchor = pin if pin is not None else last_rsq
                    if anchor is not None:
                        add_dep_helper(th_i.ins, anchor.ins,
                                       reason="batch ACT table sets")
                    th_tiles[q] = th

            def tail_step(img_state, q):
                xb, z, base, last_rsq, th_tiles = img_state
                # out = 0.2x - 0.4*th + 0.4  (th = -tanh(0.25 norm)),
                # one fused custom DVE op over flattened [P, 2W] APs.
                ot = p_out.tile([P, 2, W], bf16, tag="out")
                flat = "p a b -> p (a b)"
                nc.vector._custom_dve(
                    outaff,
                    out=ot[:].rearrange(flat),
                    in0=xb[:, 2 * q:2 * q + 2, :].rearrange(flat),
                    in1=th_tiles[q][:].rearrange(flat),
                    s0=0.2, s1=-0.4, imm2=0.4)
                nc.sync.dma_start(out=y_rows(y_ap, base + 256 * q), in_=ot[:])

            def load_img(img):
                """DMA quarters + xb = bf16 cast of x (DVE; the 0.5 for
                stage 1 lives in band_b and ident) + tb = x^2 (GPSIMD)."""
                base = img * H
                xb = p_xb.tile([P, NBLK, W], bf16, tag="xb")
                tb = p_tb.tile([P, NBLK, W], f16, tag="tb")
                for q in range(NQ):
                    xt = p_x.tile([P, 2, W], f32, tag="x_q")
                    nc.sync.dma_start(out=xt[:], in_=y_rows(x_ap, base + 256 * q))
                    # one of four casts per image goes to ACT to balance
                    # DVE (~164us busy) against ACT (~145us busy)
                    if q == 0:
                        nc.scalar.copy(out=xb[:, 2 * q: 2 * q + 2, :],
                                       in_=xt[:])
                    else:
                        nc.vector.tensor_copy(xb[:, 2 * q: 2 * q + 2, :],
                                              xt[:])
                    nc.gpsimd.tensor_tensor(
                        tb[:, 2 * q: 2 * q + 2, :], xt[:], xt[:], op=ALU.mult)
                return xb, tb

            pending = None   # tail state of previous image
            loaded = {}      # img -> (xb, tb)

            for img in range(IMGS):
                base = img * H
                last = img == IMGS - 1
                if img == 0:
                    loaded[0] = load_img(0)
                xb, tb = loaded.pop(img)

                # ---- stage 1: fused transpose+colbox for x and x^2 ----
                # [P,512] halves; evacs mostly on DVE (ACT is rsqrt/Square
                # heavy in stage 2), 1-in-6 on ACT.
                if pending is not None:
                    tail_tanh(pending, range(NQ))
                t1x = p_t1.tile([P, NBLK, W], f16, tag="t1")
                t1t = p_t1.tile([P, NBLK, W], f16, tag="t1")
                gi = 0
                for wt in range(NBLK):
                    for (dst, src_t, bnd) in ((t1x, xb, band_b),
                                              (t1t, tb, band_h)):
                        for k in (0, 1):
                            ps = ps_1.tile([P, 512], f32, tag="ps1")
                            half_mms(ps, bnd,
                                     lambda b: src_t[:, b, wt * P:(wt + 1) * P],
                                     k)
                            dcol = dst[:, wt, 512 * k:512 * (k + 1)]
                            if gi % 3 == 2:
                                nc.scalar.copy(out=dcol, in_=ps[:])
                            else:
                                nc.vector.tensor_copy(dcol, ps[:])
                            if gi % 8 == 7 and pending is not None:
                                tail_step(pending, gi // 8)
                            gi += 1
                pending = None

                # prefetch next image so stage-1(i+1) is PE-ready
                if not last:
                    loaded[img + 1] = load_img(img + 1)

                # ---- stage 2: 16 halves h = 2m+k ----------------------
                # per half: psS = 0.5*mean (5 band MMs), psQ = sqmean,
                # A = mean^2 (ACT Square from psS), then staggered PE
                # folds psS -= xb, psQ -= A; rsqrt reads var from PSUM;
                # z = psS * rc on DVE (lag 2 halves).
                z = p_z.tile([P, NBLK, W], bf16, tag="z")
                state = [xb, z, base, None, [None] * NQ]
                psS_h = [None] * 16
                psQ_h = [None] * 16
                sq_h = [None] * 16     # ACT Square insts
                a_h = [None] * 16      # mean^2 tiles
                rc_h = [None] * 16

                def emit_folds(h):
                    m, k = h // 2, h % 2
                    fn = nc.tensor.matmul(
                        psS_h[h][:], ident[:],
                        xb[:, m, 512 * k:512 * (k + 1)],
                        start=False, stop=True)
                    add_dep_helper(fn.ins, sq_h[h].ins,
                                   reason="num fold after Square read")
                    nc.tensor.matmul(
                        psQ_h[h][:], identa[:], a_h[h][:],
                        start=False, stop=True)
                    # rsqrt: var straight from PSUM; rc = 0.5/std
                    rc = p_rcp.tile([P, 512], bf16, tag="rcp", name="rc")
                    rsq_i = nc.scalar.activation(
                        rc[:], psQ_h[h][:], ACT.Abs_reciprocal_sqrt,
                        bias=0.0, scale=4.0)
                    rc_h[h] = rc
                    state[3] = rsq_i
                    psQ_h[h] = None

                def emit_z(h):
                    m, k = h // 2, h % 2
                    nc.vector.tensor_tensor(
                        z[:, m, 512 * k:512 * (k + 1)],
                        psS_h[h][:], rc_h[h][:], op=ALU.mult)
                    psS_h[h] = None

                for h in range(16):
                    m, k = h // 2, h % 2
                    psS = ps_s.tile([P, 512], f32, tag="psS")
                    half_mms(psS, band_c,
                             lambda b: t1x[:, b, m * P:(m + 1) * P], k)
                    # A = (2*psS)^2 = mean^2 (bf16 SBUF, feeds the Q fold);
                    # emitted before the Q bands so ACT has a head start
                    # and the folds never stall the PE.
                    at = p_a.tile([P, 512], bf16, tag="A")
                    sq_h[h] = nc.scalar.activation(
                        at[:], psS[:], ACT.Square, bias=0.0, scale=2.0)
                    a_h[h] = at
                    psQ = ps_q.tile([P, 512], f32, tag="psQ")
                    half_mms(psQ, band_c,
                             lambda b: t1t[:, b, m * P:(m + 1) * P], k)
                    psS_h[h], psQ_h[h] = psS, psQ
                    if h >= 1:
                        emit_folds(h - 1)
                    if h >= 2:
                        emit_z(h - 2)
                    # last image: start draining the tail early
                    if last and h == 11:
                        tail_tanh(state, (0, 1))
                        tail_step(state, 0)
                        tail_step(state, 1)
                emit_folds(15)
                emit_z(14)
                emit_z(15)
                if last:
                    tail_tanh(state, (2, 3))
                    tail_step(state, 2)
                    tail_step(state, 3)
                else:
                    pending = state

    nc.compile()
    return nc


def _get_compiled():
    global _compiled
    with _lock:
        if _compiled is None:
            band = _band_np()
            nc = _build()
            _compiled = (nc, band)
    return _compiled


def _run(x, trace=False, **kw):
    from concourse.bass_utils import run_bass_kernel_spmd

    nc, band = _get_compiled()
    band_b = np.ascontiguousarray((band * 0.5).astype(ml_dtypes.bfloat16))
    band_h = np.ascontiguousarray(band.astype(np.float16))
    band_c = np.ascontiguousarray((band * AREA_INV).astype(np.float16))
    band_q = np.ascontiguousarray(_band_q_np().astype(np.float16))
    ident = np.ascontiguousarray((-0.5 * np.eye(P, dtype=np.float32))
                                 .astype(ml_dtypes.bfloat16))
    identa = np.ascontiguousarray((-np.eye(P, dtype=np.float32))
                                  .astype(ml_dtypes.bfloat16))
    x = np.asarray(x, dtype=np.float32).reshape(B_FULL, H, W)
    core_ids = list(range(NCORES))
    in_maps = []
    for i in core_ids:
        xs = np.ascontiguousarray(
            x[IMGS * i: IMGS * (i + 1)].reshape(IMGS * H, W))
        in_maps.append({"x": xs, "bandb": band_b, "bandh": band_h,
                        "bandc": band_c, "bandq": band_q,
                        "ident": ident, "identa": identa})
    res = run_bass_kernel_spmd(nc, in_maps, core_ids, trace=trace, **kw)
    out = np.concatenate(
        [res.results[i]["y"].astype(np.float32).reshape(IMGS, 1, H, W)
         for i in core_ids], axis=0)
    return out, res


def kernel(x):
    out, _ = _run(x, trace=False)
    return out


# revision 47
# speedup vs baseline: 1.0001x; 1.0001x over previous
"""CLAHE/LCN kernel for Trainium2, 8-core data parallel (v2).

Math (per image, 31x31 'same' zero-padded box window):
    S  = box2d(x)   (sum)      Q = box2d(x^2)   (sum)
    mean = S/961, sqmean = Q/961, var = sqmean - mean^2
    out  = 0.2*x + 0.4 + 0.4*tanh(0.25*(x-mean)/sqrt(var))

Box filter on PE: image block X_b as stationary lhsT against a banded
0/1 moving operand computes the column 31-box of X, transposed. Two
fused transpose+box stages restore natural layout with no transposes.

v2 structure (vs the 284us baseline):
  - stage-2 accumulates into 512-wide single-bank PSUM tiles
    (psS bufs=3, psQ bufs=1, stage-1 ps bufs=2x2 banks = 8 banks).
  - stage-2 band carries the 1/961 scale, shared by S and Q paths:
    psS = 0.5*mean, psQ = sqmean.
  - A = mean^2 via ACT Square (the only legal PSUM reader pairing);
    then TWO PE folds with -identity stationaries: psS -= xb (so psS
    becomes 0.5*(mean-x), killing the DVE num pass) and psQ -= A (so
    psQ becomes var, killing the DVE var STT).
  - rc = rsqrt(4*var) = 0.5/std (ACT, reading var straight from PSUM).
  - z = psS * rc = -0.25*norm (DVE, PSUM operand); th = tanh(z) is
    -tanh(0.25 norm), so out = 0.2x - 0.4*th + 0.4 via one custom DVE
    op (OUTAFF: in0*s0 + in1*s1 + imm2) per quarter.
  - xb is a plain f32->bf16 cast (the 0.5 scale lives in band_b and in
    the -0.5 identity); tb = x^2 is the only GPSIMD op.
  - next image's x quarters + xb + tb are emitted before stage-2 so
    stage-1(i+1) PE work is ready the moment stage-2(i) drains.
  - folds/rsqrt staggered one half behind the bands, z two behind, so
    the PE never waits on ACT; Square is emitted before the Q bands.
Measured 249.5us (baseline 286.2us), rel err 1.15e-2 (gate 2e-2).
"""

import threading

import numpy as np
import ml_dtypes

# ---------------------------------------------------------------- constants
B_FULL = 32          # full batch
NCORES = 8
IMGS = B_FULL // NCORES  # images per core
H = W = 1024
P = 128              # partitions
NBLK = H // P        # 8 row blocks per image
NQ = 4               # quarters per image (2 row-tiles each)
KWIN = 31
HALF = KWIN // 2     # 15
AREA_INV = 1.0 / (KWIN * KWIN)  # 1/961

_lock = threading.Lock()
_compiled = None  # (nc, band_np)


def _band_spec():
    """Per h-block b: (lo, hi, offset into packed band array)."""
    spec = []
    off = 0
    for b in range(NBLK):
        lo = max(0, P * b - HALF)
        hi = min(H, P * b + P + HALF + 1)  # 128b+143
        spec.append((lo, hi, off))
        off += hi - lo
    return spec, off


def _band_np():
    spec, total = _band_spec()
    band = np.zeros((P, total), np.float32)
    for b, (lo, hi, off) in enumerate(spec):
        for h in range(P):
            gh = P * b + h
            r0 = max(lo, gh - HALF)
            r1 = min(hi, gh + HALF + 1)
            band[h, off + (r0 - lo): off + (r1 - lo)] = 1.0
    return band


def _mm_segments():
    """Stage-1 segment list for one [128, 1024] output tile, clipped to
    PSUM bank boundaries: (b, s0, s1, band_off, start, stop)."""
    spec, _ = _band_spec()
    per_bank = {0: [], 1: []}
    for b, (lo, hi, off) in enumerate(spec):
        for bank in (0, 1):
            s0 = max(lo, 512 * bank)
            s1 = min(hi, 512 * bank + 512)
            if s1 > s0:
                per_bank[bank].append((b, s0, s1, off + (s0 - lo)))
    out = []
    for bank in (0, 1):
        segs = per_bank[bank]
        for i, (b, s0, s1, boff) in enumerate(segs):
            out.append((b, s0, s1, boff, i == 0, i == len(segs) - 1))
    return out


def _seg_halves():
    """Stage-2 segments for a [128, 512] psum tile covering output cols
    [512k, 512k+512): per k a list of (b, c0, c1, band_off) with
    psum-local columns."""
    spec, _ = _band_spec()
    halves = {}
    for k in (0, 1):
        lo_k, hi_k = 512 * k, 512 * k + 512
        segs = []
        for b, (lo, hi, off) in enumerate(spec):
            s0, s1 = max(lo, lo_k), min(hi, hi_k)
            if s1 > s0:
                segs.append((b, s0 - lo_k, s1 - lo_k, off + (s0 - lo)))
        halves[k] = segs
    return halves


def _qspec():
    """Even-column (stride-2) band layout for the decimated Q path:
    per block b, (lo_e, n_e, off_q) where lo_e is the first even output
    col in the block's support and n_e the even-col count."""
    spec, _ = _band_spec()
    out = []
    off_q = 0
    for b, (lo, hi, off) in enumerate(spec):
        lo_e = lo + (lo & 1)
        n_e = (hi - lo_e + 1) // 2
        out.append((lo_e, n_e, off_q))
        off_q += n_e
    return out, off_q


def _band_q_np():
    """Packed stride-2, 1/961-scaled band for the decimated Q path."""
    band = _band_np()
    spec, _ = _band_spec()
    qspec, total = _qspec()
    bq = np.zeros((P, total), np.float32)
    for b, (lo, hi, off) in enumerate(spec):
        lo_e, n_e, off_q = qspec[b]
        bq[:, off_q:off_q + n_e] = band[:, off + (lo_e - lo): off + (hi - lo): 2]
    return bq * AREA_INV


def _seg_halves_q():
    """Decimated-Q segments: per half k a list of (b, c0, c1, boff) into
    a [P, 256] psum tile; psum-local col j <-> output col 512k + 2j."""
    spec, _ = _band_spec()
    qspec, _ = _qspec()
    halves = {}
    for k in (0, 1):
        lo_k, hi_k = 512 * k, 512 * k + 512
        segs = []
        for b, (lo, hi, off) in enumerate(spec):
            lo_e, n_e, off_q = qspec[b]
            e0 = max(lo_e, lo_k)
            e1 = min(hi, hi_k)
            if e1 > e0:
                c0 = (e0 - lo_k) // 2
                c1 = (e1 - lo_k + 1) // 2
                segs.append((b, c0, c1, off_q + (e0 - lo_e) // 2))
        halves[k] = segs
    return halves


def _register_outaff_op():
    """Custom DVE op: out = in0*s0 + in1*s1 + imm2 — the whole output
    stage (0.2x - 0.4*th + 0.4) in one bf16 2x-mode instruction."""
    import concourse.dve_ops as dve_ops
    from concourse.dve_spec import Spec, Src0, Src1, C0, C1, C2, lower
    from concourse.dve_spec import _has_src1
    from concourse.dve_uop import DveOpSpec

    name = "OUTAFF_CLAHE"
    for op in dve_ops.OPS:
        if op.name == name:
            return op
    spec = Spec(
        body=Src0 * C0 + Src1 * C1 + C2,
        reference=lambda in0, in1, s0, s1, imm2: (
            in0.astype(np.float32) * s0
            + in1.astype(np.float32) * s1 + imm2),
    )
    row = dve_ops._CUSTOM_DVE_ROW_BASE + len(dve_ops.OPS)
    shas = {}
    for ver in ("v3",):
        uops = lower(spec, ver=ver)
        shas[ver] = DveOpSpec(name=name, opcode=row, uops=uops,
                              rd1_en=_has_src1(spec)).sha(ver)
    op = dve_ops.DveOp(name, spec, subdim=False, uops_sha=shas)
    dve_ops.OPS.append(op)
    dve_ops._SUB_OPCODE_FOR_NAME[name] = row
    dve_ops.CUSTOM_DVE_SPECS[name] = op.spec
    return op


def _patch_act_tables():
    """Hollow every table set except the two this kernel uses, so the
    selector maps Copy/Abs_reciprocal_sqrt to one set and Tanh to the
    other. Dict order (set IDs) is unchanged so emitted IDs stay valid."""
    import concourse.bacc as bacc_mod
    if getattr(bacc_mod, "_clahe_tables_patched", False):
        return
    orig = bacc_mod.get_activation_tables
    keep = {"abs_reciprocal_sqrt_and_small", "silu_and_others"}

    def patched(arch):
        tabs = dict(orig(arch))
        for k in tabs:
            if k not in keep:
                tabs[k] = set()
        return tabs

    bacc_mod.get_activation_tables = patched
    bacc_mod._clahe_tables_patched = True


def y_rows(dram_ap, row0):
    """DRAM AP view [P, 2, W]: element (p, t, c) <-> dram[row0+128t+p, c]."""
    sl = dram_ap[row0: row0 + 256, :]
    return sl.rearrange("(t p) c -> p t c", p=P)


def _build():
    import concourse.bacc as bacc
    import concourse.tile as tile
    from concourse.tile import add_dep_helper
    from concourse import mybir

    _patch_act_tables()
    outaff = _register_outaff_op()

    f32 = mybir.dt.float32
    f16 = mybir.dt.float16
    bf16 = mybir.dt.bfloat16
    ALU = mybir.AluOpType
    ACT = mybir.ActivationFunctionType

    spec, band_w = _band_spec()
    halves = _seg_halves()
    halves_q = _seg_halves_q()
    _, bandq_w = _qspec()
    c = AREA_INV

    nc = bacc.Bacc("TRN2", target_bir_lowering=False, debug=False,
                   num_devices=NCORES)
    x_ext = nc.dram_tensor("x", [IMGS * H, W], f32, kind="ExternalInput")
    bandb_ext = nc.dram_tensor("bandb", [P, band_w], bf16, kind="ExternalInput")
    bandh_ext = nc.dram_tensor("bandh", [P, band_w], f16, kind="ExternalInput")
    bandc_ext = nc.dram_tensor("bandc", [P, band_w], f16, kind="ExternalInput")
    bandq_ext = nc.dram_tensor("bandq", [P, bandq_w], f16, kind="ExternalInput")
    ident_ext = nc.dram_tensor("ident", [P, P], bf16, kind="ExternalInput")
    identa_ext = nc.dram_tensor("identa", [P, P], bf16, kind="ExternalInput")
    y_ext = nc.dram_tensor("y", [IMGS * H, W], bf16, kind="ExternalOutput")
    x_ap = x_ext.ap()
    y_ap = y_ext.ap()

    with tile.TileContext(nc) as tc:
        from contextlib import ExitStack
        with ExitStack() as ctx:
            def pool(name, bufs, space="SBUF"):
                return ctx.enter_context(
                    tc.tile_pool(name=name, bufs=bufs, space=space))

            singles = pool("singles", 1)
            p_x = pool("p_x", 2)       # x quarters [P,2,W] f32
            p_xb = pool("p_xb", 2)     # 0.5x full image [P,8,W] bf16
            p_tb = pool("p_tb", 2)     # x^2 full image [P,8,W] f16
            p_t1 = pool("p_t1", 2)     # t1x/t1t [P,8,W] f16 (shared tag pool)
            p_a = pool("p_a", 3)       # mean^2 per half [P,512] bf16
            p_rcp = pool("p_rcp", 4)   # 0.5/std per half [P,512] bf16
            p_z = pool("p_z", 2)       # z full image [P,8,W] bf16
            p_thu = pool("p_thu", 4)   # tanh quarters [P,2,W] bf16
            p_out = pool("p_out", 2)   # out quarters [P,2,W] bf16
            ps_1 = pool("ps1", 2, space="PSUM")   # [P,512] = 1 bank each
            ps_s = pool("psS", 4, space="PSUM")   # [P,512]
            ps_q = pool("psQ", 2, space="PSUM")   # [P,512]

            band_b = singles.tile([P, band_w], bf16)
            band_h = singles.tile([P, band_w], f16)
            band_c = singles.tile([P, band_w], f16)
            band_q = singles.tile([P, bandq_w], f16)
            ident = singles.tile([P, P], bf16)    # diag(-0.5), num fold
            identa = singles.tile([P, P], bf16)   # diag(-1.0), A fold
            nc.sync.dma_start(out=band_b[:], in_=bandb_ext.ap())
            nc.sync.dma_start(out=band_h[:], in_=bandh_ext.ap())
            nc.sync.dma_start(out=band_c[:], in_=bandc_ext.ap())
            nc.sync.dma_start(out=band_q[:], in_=bandq_ext.ap())
            nc.sync.dma_start(out=ident[:], in_=ident_ext.ap())
            nc.sync.dma_start(out=identa[:], in_=identa_ext.ap())

            def half_mms(ps, band_sb, stat_slicer, k, seg_tab=None):
                segs = (seg_tab or halves)[k]
                n = len(segs)
                for i, (b, c0, c1, boff) in enumerate(segs):
                    nc.tensor.matmul(
                        ps[:, c0:c1],
                        stat_slicer(b),
                        band_sb[:, boff: boff + (c1 - c0)],
                        start=(i == 0), stop=(i == n - 1),
                    )

            # ---------------- per-image tail ---------------------------
            def tail_tanh(img_state, qs, pin=None):
                """tanh for quarters qs; pinned after `pin` (an ACT inst)
                to keep the two ACT table sets from thrashing."""
                xb, z, base, last_rsq, th_tiles = img_state
                for q in qs:
                    th = p_thu.tile([P, 2, W], bf16, tag="thu")
                    th_i = nc.scalar.activation(th[:], z[:, 2 * q:2 * q + 2, :],
                                                ACT.Tanh, bias=0.0, scale=1.0)
                    anchor = pin if pin is not None else last_rsq
                    if anchor is not None:
                        add_dep_helper(th_i.ins, anchor.ins,
                                       reason="batch ACT table sets")
                    th_tiles[q] = th

            def tail_step(img_state, q):
                xb, z, base, last_rsq, th_tiles = img_state
                # out = 0.2x - 0.4*th + 0.4  (th = -tanh(0.25 norm)),
                # one fused custom DVE op over flattened [P, 2W] APs.
                ot = p_out.tile([P, 2, W], bf16, tag="out")
                flat = "p a b -> p (a b)"
                nc.vector._custom_dve(
                    outaff,
                    out=ot[:].rearrange(flat),
                    in0=xb[:, 2 * q:2 * q + 2, :].rearrange(flat),
                    in1=th_tiles[q][:].rearrange(flat),
                    s0=0.2, s1=-0.4, imm2=0.4)
                nc.sync.dma_start(out=y_rows(y_ap, base + 256 * q), in_=ot[:])

            def load_img(img):
                """DMA quarters + xb = bf16 cast of x (DVE; the 0.5 for
                stage 1 lives in band_b and ident) + tb = x^2 (GPSIMD)."""
                base = img * H
                xb = p_xb.tile([P, NBLK, W], bf16, tag="xb")
                tb = p_tb.tile([P, NBLK, W], f16, tag="tb")
                for q in range(NQ):
                    xt = p_x.tile([P, 2, W], f32, tag="x_q")
                    nc.sync.dma_start(out=xt[:], in_=y_rows(x_ap, base + 256 * q))
                    nc.vector.tensor_copy(xb[:, 2 * q: 2 * q + 2, :], xt[:])
                    nc.gpsimd.tensor_tensor(
                        tb[:, 2 * q: 2 * q + 2, :], xt[:], xt[:], op=ALU.mult)
                return xb, tb

            pending = None   # tail state of previous image
            loaded = {}      # img -> (xb, tb)

            for img in range(IMGS):
                base = img * H
                last = img == IMGS - 1
                if img == 0:
                    loaded[0] = load_img(0)
                xb, tb = loaded.pop(img)

                # ---- stage 1: fused transpose+colbox for x and x^2 ----
                # [P,512] halves; evacs mostly on DVE (ACT is rsqrt/Square
                # heavy in stage 2), 1-in-3 on ACT but never in the first
                # 4 groups, and the previous image's tanh ops are spread
                # one per 8-group window (Copy lives in the tanh table set
                # too, so this costs no extra set switches) — both keep
                # the phase-boundary ACT queue from stalling the PE.
                t1x = p_t1.tile([P, NBLK, W], f16, tag="t1")
                t1t = p_t1.tile([P, NBLK, W], f16, tag="t1")
                gi = 0
                for wt in range(NBLK):
                    for (dst, src_t, bnd) in ((t1x, xb, band_b),
                                              (t1t, tb, band_h)):
                        for k in (0, 1):
                            if gi % 8 == 0 and pending is not None:
                                tail_tanh(pending, (gi // 8,))
                            ps = ps_1.tile([P, 512], f32, tag="ps1")
                            half_mms(ps, bnd,
                                     lambda b: src_t[:, b, wt * P:(wt + 1) * P],
                                     k)
                            dcol = dst[:, wt, 512 * k:512 * (k + 1)]
                            if gi % 3 == 2 and gi >= 4:
                                nc.scalar.copy(out=dcol, in_=ps[:])
                            else:
                                nc.vector.tensor_copy(dcol, ps[:])
                            if gi % 8 == 7 and pending is not None:
                                tail_step(pending, gi // 8)
                            gi += 1
                pending = None

                # prefetch next image so stage-1(i+1) is PE-ready
                if not last:
                    loaded[img + 1] = load_img(img + 1)

                # ---- stage 2: 16 halves h = 2m+k ----------------------
                # per half: psS = 0.5*mean (5 band MMs), psQ = sqmean,
                # A = mean^2 (ACT Square from psS), then staggered PE
                # folds psS -= xb, psQ -= A; rsqrt reads var from PSUM;
                # z = psS * rc on DVE (lag 2 halves).
                z = p_z.tile([P, NBLK, W], bf16, tag="z")
                state = [xb, z, base, None, [None] * NQ]
                psS_h = [None] * 16
                psQ_h = [None] * 16
                sq_h = [None] * 16     # ACT Square insts
                a_h = [None] * 16      # mean^2 tiles
                rc_h = [None] * 16

                def emit_folds(h):
                    m, k = h // 2, h % 2
                    fn = nc.tensor.matmul(
                        psS_h[h][:], ident[:],
                        xb[:, m, 512 * k:512 * (k + 1)],
                        start=False, stop=True)
                    add_dep_helper(fn.ins, sq_h[h].ins,
                                   reason="num fold after Square read")
                    nc.tensor.matmul(
                        psQ_h[h][:], identa[:], a_h[h][:],
                        start=False, stop=True)
                    # rsqrt: var straight from PSUM; rc = 0.5/std
                    rc = p_rcp.tile([P, 512], bf16, tag="rcp", name="rc")
                    rsq_i = nc.scalar.activation(
                        rc[:], psQ_h[h][:], ACT.Abs_reciprocal_sqrt,
                        bias=0.0, scale=4.0)
                    rc_h[h] = rc
                    state[3] = rsq_i
                    psQ_h[h] = None

                def emit_z(h):
                    m, k = h // 2, h % 2
                    nc.vector.tensor_tensor(
                        z[:, m, 512 * k:512 * (k + 1)],
                        psS_h[h][:], rc_h[h][:], op=ALU.mult)
                    psS_h[h] = None

                for h in range(16):
                    m, k = h // 2, h % 2
                    psS = ps_s.tile([P, 512], f32, tag="psS")
                    half_mms(psS, band_c,
                             lambda b: t1x[:, b, m * P:(m + 1) * P], k)
                    # A = (2*psS)^2 = mean^2 (bf16 SBUF, feeds the Q fold);
                    # emitted before the Q bands so ACT has a head start
                    # and the folds never stall the PE.
                    at = p_a.tile([P, 512], bf16, tag="A")
                    sq_h[h] = nc.scalar.activation(
                        at[:], psS[:], ACT.Square, bias=0.0, scale=2.0)
                    a_h[h] = at
                    psQ = ps_q.tile([P, 512], f32, tag="psQ")
                    half_mms(psQ, band_c,
                             lambda b: t1t[:, b, m * P:(m + 1) * P], k)
                    psS_h[h], psQ_h[h] = psS, psQ
                    if h >= 1:
                        emit_folds(h - 1)
                    if h >= 2:
                        emit_z(h - 2)
                    # last image: start draining the tail early
                    if last and h == 11:
                        tail_tanh(state, (0, 1))
                        tail_step(state, 0)
                        tail_step(state, 1)
                emit_folds(15)
                emit_z(14)
                emit_z(15)
                if last:
                    tail_tanh(state, (2, 3))
                    tail_step(state, 2)
                    tail_step(state, 3)
                else:
                    pending = state

    nc.compile()
    return nc


def _get_compiled():
    global _compiled
    with _lock:
        if _compiled is None:
            band = _band_np()
            nc = _build()
            _compiled = (nc, band)
    return _compiled


def _run(x, trace=False, **kw):
    from concourse.bass_utils import run_bass_kernel_spmd

    nc, band = _get_compiled()
    band_b = np.ascontiguousarray((band * 0.5).astype(ml_dtypes.bfloat16))
    band_h = np.ascontiguousarray(band.astype(np.float16))
    band_c = np.ascontiguousarray((band * AREA_INV).astype(np.float16))
    band_q = np.ascontiguousarray(_band_q_np().astype(np.float16))
    ident = np.ascontiguousarray((-0.5 * np.eye(P, dtype=np.float32))
                                 .astype(ml_dtypes.bfloat16))
    identa = np.ascontiguousarray((-np.eye(P, dtype=np.float32))
                                  .astype(ml_dtypes.bfloat16))
    x = np.asarray(x, dtype=np.float32).reshape(B_FULL, H, W)
    core_ids = list(range(NCORES))
    in_maps = []
    for i in core_ids:
        xs = np.ascontiguousarray(
            x[IMGS * i: IMGS * (i + 1)].reshape(IMGS * H, W))
        in_maps.append({"x": xs, "bandb": band_b, "bandh": band_h,
                        "bandc": band_c, "bandq": band_q,
                        "ident": ident, "identa": identa})
    res = run_bass_kernel_spmd(nc, in_maps, core_ids, trace=trace, **kw)
    out = np.concatenate(
        [res.results[i]["y"].astype(np.float32).reshape(IMGS, 1, H, W)
         for i in core_ids], axis=0)
    return out, res


def kernel(x):
    out, _ = _run(x, trace=False)
    return out
